# revision 18
# baseline (speedup 1.0000x reference)
"""DistMult scoring kernel for Trainium2 (8 NeuronCores, SPMD batch-parallel).

score = sigmoid(sum_d ent[h]_d * rel[r]_d * ent[t]_d)

v2 — transfer-optimized. The axon tunnel (~70 MB/s aggregate H2D) dominated
v1, which replicated the 512 MB fp32 ent_emb to all 8 cores (4 GB/call).
Now:
- ent_emb is quantized host-side to int16 with one global scale
  (q = rint(ent/s), s = absmax/32767; max rel err vs fp32 reference ~2e-3,
  well under the 2e-2 gate). The s^2 factor is folded into rel_emb.
- Only rows actually referenced by batch_h/batch_t are shipped (~86.5% of
  the table for this batch size); indices are remapped host-side.
- Each core uploads only its 1/8 row-shard of the deduped table; an
  on-device AllGather rebuilds the full int16 table in DRAM.
  Total tunnel traffic/call: ~235 MB vs 4 GB.
- batch_r indices upload as the 16-row dma_gather wrap ([16, cols*8] int16)
  and are replicated to 128 partitions on device (3 doubling SBUF copies).

Per-core kernel (raw bass, manual semaphores):
- ent rows are fetched from the AllGather output with [P,1] indirect DMAs
  (128 rows x 256 B per instruction).
- rel rows (< 500, fits int16) are fetched with dma_gather: indices
  pre-wrapped on the host into the Q7 layout.
- Gather completion is detected with a flush barrier: a tiny normal SWDGE
  DMA on the same qPoolDynamic queue. Per-engine descriptor FIFOs guarantee
  it lands after every prior gather descriptor; its semaphore increments by
  exactly 16. (The increments attached to the gather instructions
  themselves fire early on HW — do not gate on them.)
- DVE casts h,t int16->fp32, computes prod = h*t*r and a segmented 128-wide
  reduction; ACT applies the sigmoid; one full-rate DMA writes the scores.
"""
import os

os.environ.setdefault("NEURON_RT_RESET_CORES", "1")

import numpy as np
import concourse.bacc as bacc
import concourse.bass as bass
from concourse import mybir
from concourse.bass_utils import run_bass_kernel_spmd

N_CORES = 8
P, D = 128, 128
B = 1_048_576
B_CORE = B // N_CORES            # 131072 triples per core
COLS = B_CORE // P               # 1024 triples per partition
K = 8                            # columns per super-tile (1024 triples)
N_SUPER = COLS // K
ENT = 1_000_000
REL = 500
N_BUFS = 2
N_QUEUES = 4     # SWDGE queues; each is serviced by its own Q7 core pair

_CACHED_NC = {}


def _set_queue(inst, q):
    if q:
        inst.ins.queue = f"qPoolDynamic{q}"
    return inst


def _build_nc(ent_n=ENT, rel_n=REL, cols=COLS, k=K, n_bufs=N_BUFS, n_queues=N_QUEUES):
    assert cols % k == 0 and k % n_queues == 0
    n_super = cols // k
    nc = bacc.Bacc(num_swdge_queues=n_queues, num_devices=N_CORES)
    # ALL per-core inputs byte-packed into ONE int16 tensor: each separate
    # input array costs ~60 ms of serial tunnel overhead per call.
    # int16-rows: [0:shard)=ent table shard, then bh int32 (2048 rows),
    # bt int32 (2048), br16 wrap (1024), rel fp32 (2*rel_n).
    shard = ent_n // N_CORES
    n_bh = P * cols * 2 // 128      # 2048 int16-rows for each of bh/bt
    n_br = 16 * cols * 8 // 128     # 1024 int16-rows for the br16 wrap
    aux16 = 2 * n_bh + n_br + 2 * rel_n
    payload = nc.dram_tensor(
        "payload", [shard + aux16, 128], mybir.dt.int16, kind="ExternalInput")
    a0 = shard
    bh_v = payload[a0:a0 + n_bh, :].bitcast(
        mybir.dt.int32).rearrange("(p a) b -> p a b", p=P)
    bt_v = payload[a0 + n_bh:a0 + 2 * n_bh, :].bitcast(
        mybir.dt.int32).rearrange("(p a) b -> p a b", p=P)
    br_v = payload[a0 + 2 * n_bh:a0 + 2 * n_bh + n_br, :].rearrange(
        "(p a) b -> p a b", p=16)
    rel_v = payload[a0 + 2 * n_bh + n_br:a0 + aux16, :].rearrange(
        "(r a) b -> r (a b)", a=2).bitcast(mybir.dt.float32)
    # collectives cannot read IO tensors: stage the shard into Internal DRAM
    ent_stage = nc.dram_tensor("ent_stage", [ent_n // N_CORES, D], mybir.dt.int16)
    ent_full = nc.dram_tensor("ent_full", [ent_n, D], mybir.dt.int16,
                              addr_space="Shared")
    score = nc.dram_tensor("score", [P, cols], mybir.dt.float32, kind="ExternalOutput")

    n_idx = 128 * k

    from contextlib import ExitStack
    with ExitStack() as ctx:
        h_idx = ctx.enter_context(nc.sbuf_tensor("h_idx", [P, cols], mybir.dt.int32))
        t_idx = ctx.enter_context(nc.sbuf_tensor("t_idx", [P, cols], mybir.dt.int32))
        r_idx = ctx.enter_context(nc.sbuf_tensor("r_idx", [128, cols * 8], mybir.dt.int16))
        scores = ctx.enter_context(nc.sbuf_tensor("scores", [P, cols], mybir.dt.float32))
        sig = ctx.enter_context(nc.sbuf_tensor("sig", [P, cols], mybir.dt.float32))
        flush_a = ctx.enter_context(nc.sbuf_tensor("flush_a", [P, n_queues], mybir.dt.float32))
        flush_b = ctx.enter_context(nc.sbuf_tensor("flush_b", [P, n_queues], mybir.dt.float32))
        h_buf = ctx.enter_context(nc.sbuf_tensor("h_buf", [P, n_bufs * k * D], mybir.dt.int16))
        t_buf = ctx.enter_context(nc.sbuf_tensor("t_buf", [P, n_bufs * k * D], mybir.dt.int16))
        r_buf = ctx.enter_context(nc.sbuf_tensor("r_buf", [P, n_bufs * k * D], mybir.dt.float32))
        hf = ctx.enter_context(nc.sbuf_tensor("hf", [P, k * D], mybir.dt.float32))
        tf = ctx.enter_context(nc.sbuf_tensor("tf", [P, k * D], mybir.dt.float32))
        i_sem = ctx.enter_context(nc.semaphore("i_sem"))
        r_sem = ctx.enter_context(nc.semaphore("r_sem"))
        cc_sem = ctx.enter_context(nc.semaphore("cc_sem"))
        gh_sem = ctx.enter_context(nc.semaphore("gh_sem"))
        gt_sem = ctx.enter_context(nc.semaphore("gt_sem"))
        gr_sem = ctx.enter_context(nc.semaphore("gr_sem"))
        f_sem = ctx.enter_context(nc.semaphore("f_sem"))
        v_sem = ctx.enter_context(nc.semaphore("v_sem"))
        s_sem = ctx.enter_context(nc.semaphore("s_sem"))
        o_sem = ctx.enter_context(nc.semaphore("o_sem"))
        block = ctx.enter_context(nc.Block())
        def bufsl(buf, s, j=None):
            b = s % n_bufs
            if j is None:
                return buf[:, b * k * D:(b + 1) * k * D]
            return buf[:, (b * k + j) * D:(b * k + j + 1) * D]

        @block.sync
        def _(sync):
            sync.dma_start(out=ent_stage[:], in_=payload[0:shard, :]).then_inc(cc_sem, 16)
            sync.dma_start(
                out=h_idx[:].rearrange("p (a b) -> p a b", b=64), in_=bh_v
            ).then_inc(i_sem, 16)
            sync.dma_start(
                out=t_idx[:].rearrange("p (a b) -> p a b", b=64), in_=bt_v
            ).then_inc(i_sem, 16)
            # 16-row dma_gather index wrap -> replicate to all 128 partitions
            sync.dma_start(
                out=r_idx[0:16, :].rearrange("p (a b) -> p a b", b=128), in_=br_v
            ).then_inc(r_sem, 16)
            sync.wait_ge(r_sem, 16)
            sync.dma_start(out=r_idx[16:32, :], in_=r_idx[0:16, :]).then_inc(r_sem, 16)
            sync.wait_ge(r_sem, 32)
            sync.dma_start(out=r_idx[32:64, :], in_=r_idx[0:32, :]).then_inc(r_sem, 16)
            sync.wait_ge(r_sem, 48)
            sync.dma_start(out=r_idx[64:128, :], in_=r_idx[0:64, :]).then_inc(r_sem, 16)
            sync.wait_ge(s_sem, 1)
            sync.dma_start(out=score[:], in_=sig[:]).then_inc(o_sem, 16)

        @block.gpsimd
        def _(g):
            g.wait_ge(cc_sem, 16)
            g.collective_compute(
                "AllGather", mybir.AluOpType.bypass,
                replica_groups=[list(range(N_CORES))],
                ins=[ent_stage[:].opt()], outs=[ent_full[:].opt()],
            ).then_inc(cc_sem, 1)
            g.wait_ge(cc_sem, 17)
            g.wait_ge(i_sem, 32)
            g.wait_ge(r_sem, 64)
            for s in range(n_super):
                if s >= n_bufs:
                    g.wait_ge(v_sem, s - n_bufs + 1)
                for j in range(k):
                    col = s * k + j
                    q = j % n_queues
                    _set_queue(g.indirect_dma_start(
                        out=bufsl(h_buf, s, j), out_offset=None, in_=ent_full[:],
                        in_offset=bass.IndirectOffsetOnAxis(
                            ap=h_idx[:, col:col + 1], axis=0),
                    ), q).then_inc(gh_sem, 16)
                    _set_queue(g.indirect_dma_start(
                        out=bufsl(t_buf, s, j), out_offset=None, in_=ent_full[:],
                        in_offset=bass.IndirectOffsetOnAxis(
                            ap=t_idx[:, col:col + 1], axis=0),
                    ), q).then_inc(gt_sem, 16)
                g.dma_gather(
                    out_ap=bufsl(r_buf, s).rearrange("p (c d) -> p c d", d=D),
                    in_ap=rel_v,
                    idxs_ap=r_idx[:, s * 8 * k:(s + 1) * 8 * k],
                    num_idxs=n_idx,
                    num_idxs_reg=n_idx,
                    elem_size=D,
                ).then_inc(gr_sem, 16)
                for q in range(n_queues):
                    _set_queue(
                        g.dma_start(out=flush_b[:, q:q + 1],
                                    in_=flush_a[:, q:q + 1]),
                        q,
                    ).then_inc(f_sem, 16)

        @block.vector
        def _(v):
            for s in range(n_super):
                ksl = slice(s * k, (s + 1) * k)
                h_sl, t_sl, r_sl = bufsl(h_buf, s), bufsl(t_buf, s), bufsl(r_buf, s)
                v.wait_ge(f_sem, 16 * n_queues * (s + 1))
                v.tensor_copy(out=hf[:], in_=h_sl)
                v.tensor_copy(out=tf[:], in_=t_sl)
                v.tensor_mul(out=hf[:], in0=hf[:], in1=tf[:])
                v.tensor_mul(out=hf[:], in0=hf[:], in1=r_sl)
                v.tensor_reduce(
                    out=scores[:, ksl],
                    in_=hf[:].rearrange("p (k d) -> p k d", d=D),
                    axis=mybir.AxisListType.X,
                    op=mybir.AluOpType.add,
                ).then_inc(v_sem, 1)

        @block.scalar
        def _(a):
            a.wait_ge(v_sem, n_super)
            a.activation(
                out=sig[:], in_=scores[:],
                func=mybir.ActivationFunctionType.Sigmoid,
            ).then_inc(s_sem, 1)

    nc.compile()
    return nc


def _wrap_r16(r2d, k=K):
    """[P, cols] ints -> [16, cols*8] int16 dma_gather index layout.

    Super-tile s, gather list position j = c*128 + p <-> triple (p, s*k+c);
    int16 value sits at [j % 16, s*8*k + j//16]. The 16-row pattern is
    replicated down to all 128 partitions on device.
    """
    p_, cols = r2d.shape
    assert p_ == P and cols % k == 0
    out = np.empty((16, cols * 8), np.int16)
    for s in range(cols // k):
        blk = r2d[:, s * k:(s + 1) * k]
        lst = blk.T.reshape(-1)
        out[:, s * 8 * k:(s + 1) * 8 * k] = lst.astype(np.int16).reshape(-1, 16).T
    return out


def _get_nc(ent_n=ENT):
    if ent_n not in _CACHED_NC:
        _CACHED_NC[ent_n] = _build_nc(ent_n=ent_n)
    return _CACHED_NC[ent_n]


def make_in_maps(batch_h, batch_t, batch_r, ent_emb, rel_emb):
    bh = np.asarray(batch_h).astype(np.int32).reshape(B)
    bt = np.asarray(batch_t).astype(np.int32).reshape(B)
    br = np.asarray(batch_r).astype(np.int32).reshape(B)
    ent = np.asarray(ent_emb, dtype=np.float32)
    rel = np.asarray(rel_emb, dtype=np.float32)
    # keep only rows actually referenced; remap indices into the deduped table
    uniq = np.unique(np.concatenate([bh, bt]))
    bh = np.searchsorted(uniq, bh).astype(np.int32)
    bt = np.searchsorted(uniq, bt).astype(np.int32)
    # int16 global-scale quantization; fold scale^2 into the rel table
    amax = float(np.abs(ent).max())
    scale = max(amax, 1e-30) / 32767.0
    ent16 = np.clip(np.rint(ent[uniq] * (1.0 / scale)), -32767, 32767).astype(np.int16)
    rel_s = np.ascontiguousarray(rel * np.float32(scale * scale))
    shard = -(-len(uniq) // N_CORES)           # ceil; gathered table = 8*shard rows
    ent_n = shard * N_CORES
    if len(ent16) < ent_n:
        ent16 = np.concatenate(
            [ent16, np.zeros((ent_n - len(ent16), D), np.int16)])
    rel_rows = rel_s.view(np.int16).reshape(-1, 128)
    in_maps = []
    for c in range(N_CORES):
        sl = slice(c * B_CORE, (c + 1) * B_CORE)
        payload = np.concatenate([
            ent16[c * shard:(c + 1) * shard],
            bh[sl].view(np.int16).reshape(-1, 128),
            bt[sl].view(np.int16).reshape(-1, 128),
            _wrap_r16(br[sl].reshape(P, COLS)).reshape(-1, 128),
            rel_rows,
        ])
        in_maps.append({"payload": payload})
    return in_maps, ent_n


def kernel(batch_h, batch_t, batch_r, ent_emb, rel_emb, **_):
    in_maps, ent_n = make_in_maps(batch_h, batch_t, batch_r, ent_emb, rel_emb)
    nc = _get_nc(ent_n)
    res = None
    last_err = None
    for _attempt in range(3):
        try:
            res = run_bass_kernel_spmd(nc, in_maps, list(range(N_CORES)))
            break
        except Exception as e:  # transient NRT device resets on first load
            last_err = e
    if res is None:
        raise last_err
    return np.concatenate(
        [res.results[c]["score"].reshape(B_CORE) for c in range(N_CORES)]
    )
